# revision 1
# baseline (speedup 1.0000x reference)
"""Multi-head causal attention (B=2, S=2048, D=1024, H=16) on 8 TRN2 NeuronCores.

Sharding: core c -> batch c//4, head-quarter c%4 (4 heads = 256 head dims).
Each core runs the full pipeline for its (batch, 4 heads); host sums the 4
row-sharded out-projection partials per batch + bias.

v2 design (PE-column-minimal, all matmuls bf16 at rate 1 col/cycle):
  - QKV projections: moving xT [128,512] bf16, 8-chunk contraction.
  - Scores: per 128-k-tile, live-q-trimmed moving qT (free 512-coff).
  - exp on Act into bf16 `at` tiles; causal diag masked by tri-mult on Pool.
  - PV in stationary-attention layout: out[q,65] += at[k,q].T @ [v|1][k,65]
    so each k-tile costs only 65 PE columns; all 4 heads accumulate in ONE
    PSUM bank (DVE memset pre-zeros the bank, every matmul start=False -
    hardware start_tensor_calc only zeroes bytes the matmul itself writes).
  - rowsum rides along as the ones column; normalize = per-partition
    tensor_scalar (DVE early, Act Copy+scale late), bf16 ctx; XBAR
    DMA-transpose back to d-major cT for the out-projection.
  - out-projection from cT bf16; both halves staged into one [128,1024]
    out DMA; PSUM->SBUF copies on DVE so Act stays exp-only.
Emission is software-pipelined by a FIFO fill queue paced to the Act
engine's exp rate: projection groups, PV blocks and out-proj tiles drain
into the scores i-loop's Act-bound surplus, with out-proj units for the
first 12 row-tiles hoarded for chunk 3's long exp-only stretch.
"""

import sys

import numpy as np

if "/opt/trn_rl_repo" not in sys.path:
    sys.path.insert(0, "/opt/trn_rl_repo")

import concourse.bass as bass
import concourse.mybir as mybir
import concourse.tile as tile
from concourse.bass import ts
from concourse.bass_utils import run_bass_kernel_spmd

P = 128          # partitions
S = 2048         # sequence length
DD = 1024        # model dim
DC = DD // P     # d-model chunks (8)
E = 256          # head dims per core (4 heads x 64)
H4 = 4           # heads per core
HD = 64
NQ = 4           # q chunks of 512
QC = 512
KT = S // P      # k tiles (16)
FD = 512         # out-proj free dim

F32 = mybir.dt.float32
BF16 = mybir.dt.bfloat16
EXP = mybir.ActivationFunctionType.Exp
MUL = mybir.AluOpType.mult


def _emit(tc, nc, xT_d, wq_d, wk_d, wv_d, wo_d, tri_d, out_d):
    with (
        tc.tile_pool(name="const", bufs=1) as const,
        tc.tile_pool(name="attn", bufs=53) as attn_pool,
        tc.tile_pool(name="small", bufs=4) as small,
        tc.tile_pool(name="ctxp", bufs=4) as ctxp,
        tc.tile_pool(name="ostage", bufs=3) as ostage,
        tc.tile_pool(name="pmm", bufs=2, space="PSUM") as pmm,
        tc.tile_pool(name="pacc", bufs=2, space="PSUM") as pacc,
        tc.tile_pool(name="psc", bufs=2, space="PSUM") as psc,
    ):
        xT = const.tile([P, DC, S], BF16)
        wq = const.tile([P, DC, E], BF16)
        wk = const.tile([P, DC, E], BF16)
        wv = const.tile([P, DC, E], BF16)
        wo = const.tile([P, 2, DD], BF16)
        tri = const.tile([P, P], BF16)
        qT = const.tile([P, 2, S], BF16)
        kT = const.tile([P, 2, S], BF16)
        vS = const.tile([P, KT, H4, HD + 1], BF16)
        cT = const.tile([P, 2, S], BF16)

        # j=0 slices of x^T first so the first projections can start early;
        # issue across both HWDGE queues (SP + Act) to halve serialization
        nc.scalar.dma_start(xT[:, 0:1, 0:QC], xT_d[:, 0:1, 0:QC])
        nc.sync.dma_start(wq[:, 0:1, :], wq_d[:, 0:1, :])
        nc.sync.dma_start(wq[:, 1:2, :], wq_d[:, 1:2, :])
        nc.sync.dma_start(xT[:, 1:2, 0:QC], xT_d[:, 1:2, 0:QC])
        nc.scalar.dma_start(xT[:, 2:3, 0:QC], xT_d[:, 2:3, 0:QC])
        nc.sync.dma_start(wq[:, 2:8, :], wq_d[:, 2:8, :])
        nc.scalar.dma_start(xT[:, 3:5, 0:QC], xT_d[:, 3:5, 0:QC])
        nc.sync.dma_start(xT[:, 5:7, 0:QC], xT_d[:, 5:7, 0:QC])
        nc.scalar.dma_start(wk[:, 0:4, :], wk_d[:, 0:4, :])
        nc.sync.dma_start(xT[:, 7:8, 0:QC], xT_d[:, 7:8, 0:QC])
        nc.scalar.dma_start(wk[:, 4:8, :], wk_d[:, 4:8, :])
        nc.scalar.dma_start(wv[:], wv_d[:])
        nc.sync.dma_start(tri[:], tri_d[:])
        nc.sync.dma_start(xT[:, 0:4, QC:S], xT_d[:, 0:4, QC:S])
        nc.scalar.dma_start(xT[:, 4:8, QC:S], xT_d[:, 4:8, QC:S])
        nc.sync.dma_start(wo[:], wo_d[:])

        # ones column of [V|1] (rowsums of masked exp-scores come out of PV)
        nc.vector.memset(vS[:, :, :, HD], 1.0)

        def psum_copy(dst, src):
            # all PSUM->SBUF copies on VectorE; ScalarE stays exp-only
            nc.vector.tensor_copy(dst, src)

        def emit_qk_proj(j):
            for w_s, dst in ((wq, qT), (wk, kT)):
                for et in range(2):
                    ps = pmm.tile([P, QC], F32, tag="mm", name="ps_proj")
                    for c in range(DC):
                        nc.tensor.matmul(
                            ps[:],
                            lhsT=w_s[:, c, ts(et, P)],
                            rhs=xT[:, c, ts(j, QC)],
                            start=(c == 0),
                            stop=(c == DC - 1),
                        )
                    psum_copy(dst[:, et, ts(j, QC)], ps[:])

        def emit_v_proj(nt):
            psv = pmm.tile([P, E], F32, tag="mm", name="ps_v")
            for c in range(DC):
                nc.tensor.matmul(
                    psv[:],
                    lhsT=xT[:, c, ts(nt, P)],
                    rhs=wv[:, c, :],
                    start=(c == 0),
                    stop=(c == DC - 1),
                )
            psum_copy(
                vS[:, nt, :, 0:HD],
                psv[:].rearrange("p (h d) -> p h d", h=H4),
            )

        def emit_scores(j, i, at_tiles):
            # scores + exp for k-tile i against q-chunk j, both head pairs
            coff = max(0, P * (i - 4 * j))
            for hp in range(2):
                sc = psc.tile([P, 2, QC], F32, tag="sc", name="sc")
                at = attn_pool.tile([P, 2, QC], BF16, tag="at", name="at")
                for hh in range(2):
                    po = HD * hh
                    nc.tensor.matmul(
                        sc[:, hh, coff:QC],
                        lhsT=kT[po:po + HD, hp, ts(i, P)],
                        rhs=qT[po:po + HD, hp, j * QC + coff:(j + 1) * QC],
                        start=True,
                        stop=True,
                    )
                nc.scalar.activation(at[:, :, coff:QC], sc[:, :, coff:QC], EXP)
                if i >= 4 * j:  # diagonal 128x128 block: causal triangle
                    for hh in range(2):
                        nc.gpsimd.tensor_tensor(
                            at[:, hh, coff:coff + P],
                            at[:, hh, coff:coff + P],
                            tri[:],
                            MUL,
                        )
                at_tiles[i, hp] = at

        def emit_pv(j, qs, at_tiles, split=False):
            # ctx[q, d] for global q-subtile qs, all 4 heads in one PSUM bank.
            # split=True normalizes/transposes per head-pair (shorter drain
            # chain) - worth it only for the final subtile's tail.
            qo = P * (qs - 4 * j)
            # explicit zero-init (hardware start_tensor_calc only overwrites
            # bytes each matmul writes, so the four head-groups sharing one
            # bank must accumulate start=False onto real zeros). Tiles are
            # pre-staged so the next PV's memset sits ahead of this PV's
            # normalize ops in the DVE queue.
            if pacc_pre:
                pv = pacc_pre.pop()
            else:
                pv = pacc.tile([P, H4, HD + 1], F32, tag="pv", name="pv")
                nc.vector.memset(pv[:], 0.0)
            cq = ctxp.tile([P, H4, HD], BF16, tag="cq", name="cq")

            def normalize(hp):
                lo = 2 * hp if split else 0
                hi = 2 * hp + 2 if split else H4
                rec = small.tile([P, hi - lo], F32, tag="rec", name="rec")
                nc.vector.reciprocal(rec[:], pv[:, lo:hi, HD])
                for h in range(lo, hi):
                    if qs >= 13:  # Act is past its exp backlog by then
                        nc.scalar.activation(
                            cq[:, h, :], pv[:, h, 0:HD],
                            mybir.ActivationFunctionType.Copy,
                            scale=rec[:, h - lo:h - lo + 1],
                        )
                    else:
                        nc.vector.tensor_scalar(
                            cq[:, h, :], pv[:, h, 0:HD], rec[:, h - lo:h - lo + 1],
                            None, MUL,
                        )
                for half in range(hp, hp + 1) if split else range(2):
                    nc.sync.dma_start(
                        out=cT[:, half, ts(qs, P)],
                        in_=cq[:, 2 * half:2 * half + 2, :],
                        transpose=True,
                    )

            for hp in range(2):
                for hh in range(2):
                    h = 2 * hp + hh
                    for i in range(qs + 1):
                        nc.tensor.matmul(
                            pv[:, h, :],
                            lhsT=at_tiles[i, hp][:, hh, qo:qo + P],
                            rhs=vS[:, i, h, :],
                            start=False,
                            stop=(hh == 1 and i == qs),
                            skip_group_check=True,
                        )
                if split:
                    if hp == 1:
                        prestage_pacc()
                    normalize(hp)
            if not split:
                prestage_pacc()
                normalize(0)

        ob_half = {}
        pacc_pre = []
        pv_left = [4 * KT // 4]  # 16 PV subtiles total

        def prestage_pacc():
            pv_left[0] -= 1
            if pv_left[0] > 0 and not pacc_pre:
                t = pacc.tile([P, H4, HD + 1], F32, tag="pv", name="pv")
                nc.vector.memset(t[:], 0.0)
                pacc_pre.append(t)

        def emit_out_proj(nt, fc, tail=False):
            po = pmm.tile([P, FD], F32, tag="mm", name="ps_out")
            for c in range(2):
                nc.tensor.matmul(
                    po[:],
                    lhsT=cT[:, c, ts(nt, P)],
                    rhs=wo[:, c, ts(fc, FD)],
                    start=(c == 0),
                    stop=(c == 1),
                )
            if tail:
                # tail latency: separate half-DMAs, copies on idle Act,
                # issues alternating between the two HWDGE queues
                ob = ostage.tile([P, FD], F32, tag="obt", name="obt")
                if fc == 0:  # parallel tail copies on Act and DVE
                    nc.scalar.copy(ob[:], po[:])
                    nc.sync.dma_start(out_d[ts(nt, P), ts(fc, FD)], ob[:])
                else:
                    nc.vector.tensor_copy(ob[:], po[:])
                    nc.scalar.dma_start(out_d[ts(nt, P), ts(fc, FD)], ob[:])
                return
            # merge both halves of a row-tile into one staging tile and a
            # single [128, 1024] DMA: halves the SP issue cost
            if fc == 0:
                ob_half[nt] = ostage.tile([P, 2, FD], F32, tag="ob", name="ob")
            ob = ob_half[nt]
            if nt >= 13 and fc == 1:  # Act has slack in the wind-down
                nc.scalar.copy(ob[:, fc, :], po[:])
            else:
                psum_copy(ob[:, fc, :], po[:])
            if fc == 1:
                nc.sync.dma_start(out_d[ts(nt, P), :], ob_half.pop(nt)[:])

        # Global software pipeline. The scores i-loop is Act(exp)-bound, so
        # every other PE unit (projection groups, PV blocks, out-proj tiles)
        # goes into a FIFO fill queue drained per-iteration by the Act-pace
        # surplus. Un-drained units carry across chunk boundaries, which
        # automatically defers PV/out-proj work into chunk 3's long
        # exp-only stretch.
        at_tiles = {}
        fill = []  # (cost_ns, kind, emit_fn)

        pace = [0.0]  # cumulative Act-surplus vs drained PE cost

        def drain(budget):
            # surplus while the queue is empty is wasted, not banked
            pace[0] = min(pace[0] + budget, 4000.0)
            while fill and pace[0] > 0:
                cost, _, fn = fill.pop(0)
                fn()
                pace[0] -= cost

        def force_drain(kind):
            # emit every queued unit up to and including the last of `kind`
            last = max((n for n, (_, k, _) in enumerate(fill) if k == kind),
                       default=-1)
            for _ in range(last + 1):
                _, _, fn = fill.pop(0)
                fn()

        deferred_out = []  # out-proj units hoarded for chunk 3's exp stretch
        in_last = [False]

        def push_pv(j, qs):
            cost = (qs + 1) * H4 * (HD + 1) * 0.42 + 100
            snap = dict(at_tiles)  # at_tiles mutates before deferred drain
            def go():
                emit_pv(j, qs, snap, split=True)
                dst = fill if (in_last[0] or qs >= 12) else deferred_out
                tail = qs >= 15
                dst.append((430, "out", lambda: emit_out_proj(qs, 0, tail)))
                dst.append((430, "out", lambda: emit_out_proj(qs, 1, tail)))
            fill.append((cost, "pv", go))

        emit_qk_proj(0)
        for nt in range(4):
            emit_v_proj(nt)

        for j in range(NQ):
            if j + 1 < NQ:
                fill.append((6900, f"qk{j + 1}", lambda j1=j + 1: emit_qk_proj(j1)))
                for nt in range(4 * (j + 1), 4 * (j + 1) + 4):
                    fill.append((880, f"v{j + 1}", lambda nt=nt: emit_v_proj(nt)))
            if j > 0:
                force_drain(f"qk{j}")  # this chunk's q/k must exist
            if j == NQ - 1:
                in_last[0] = True
                fill.extend(deferred_out)
                deferred_out.clear()
            nk = 4 * (j + 1)
            for i in range(nk):
                if i == 4 * j and j > 0:
                    force_drain(f"v{j}")  # PV(j, 4j) is imminent
                emit_scores(j, i, at_tiles)
                # Act per iter: 2 exps of 2*(QC-coff) free each + overhead;
                # scores PE cost: 4 matmuls of (QC-coff) cols. Drain just
                # under the surplus so Act never waits on the next scores.
                w = QC - max(0, P * (i - 4 * j))
                drain(1.46 * w + 400)
                if i > 4 * j:
                    push_pv(j, i - 1)
            push_pv(j, 4 * j + 3)
        # final flush: the last PV goes first so its normalize/transpose
        # chain drains while PE chews the remaining out-proj tiles
        fill.sort(key=lambda u: 0 if u[1] == "pv" else 1)
        while fill:
            _, _, fn = fill.pop(0)
            fn()


def _split_multi_waits(nc):
    """The TRN2 instruction encoding carries ONE sync-wait slot; this walrus
    build rejects instructions with more. Hoist extra waits onto standalone
    EventSemaphore instructions immediately before (same engine queue, same
    semantics)."""
    n = 0
    for f in nc.m.functions:
        for b in f.blocks:
            out = []
            for i in list(b.instructions):
                si = i.sync_info
                if si is not None and len(si.on_wait) > 1:
                    waits = list(si.on_wait)
                    for w in waits[:-1]:
                        n += 1
                        out.append(
                            mybir.InstEventSemaphore(
                                name=f"I-wsplit{n}",
                                engine=i.engine,
                                ins=[],
                                outs=[],
                                sync_info=mybir.SyncInfo(on_wait=[w], on_update=[]),
                            )
                        )
                    i.sync_info = mybir.SyncInfo(
                        on_wait=[waits[-1]], on_update=list(si.on_update)
                    )
                out.append(i)
            b.instructions = out


def build_nc(split_waits=True):
    nc = bass.Bass("TRN2", target_bir_lowering=False, debug=False)
    xT_d = nc.dram_tensor("xT", [P, DC, S], BF16, kind="ExternalInput").ap()
    wq_d = nc.dram_tensor("wqT", [P, DC, E], BF16, kind="ExternalInput").ap()
    wk_d = nc.dram_tensor("wkT", [P, DC, E], BF16, kind="ExternalInput").ap()
    wv_d = nc.dram_tensor("wvT", [P, DC, E], BF16, kind="ExternalInput").ap()
    wo_d = nc.dram_tensor("woT", [P, 2, DD], BF16, kind="ExternalInput").ap()
    tri_d = nc.dram_tensor("tri", [P, P], BF16, kind="ExternalInput").ap()
    out_d = nc.dram_tensor("out", [S, DD], F32, kind="ExternalOutput").ap()
    with tile.TileContext(nc) as tc:
        _emit(tc, nc, xT_d, wq_d, wk_d, wv_d, wo_d, tri_d, out_d)
    if split_waits:
        _split_multi_waits(nc)
    return nc


def _strip(a, chunks):
    """[D, N] -> [128, D//128, N] with partition-major layout, contiguous."""
    import ml_dtypes

    d, n = a.shape
    return np.ascontiguousarray(
        a.reshape(chunks, P, n).transpose(1, 0, 2).astype(ml_dtypes.bfloat16)
    )


def make_in_maps(x, Wq, Wk, Wv, Wo):
    import ml_dtypes

    tri = np.ascontiguousarray(
        np.triu(np.ones((P, P), np.float32)).astype(ml_dtypes.bfloat16)
    )
    in_maps = []
    for c in range(8):
        b, g = c // 4, c % 4
        sl = slice(E * g, E * (g + 1))
        in_maps.append(
            {
                "xT": _strip(x[b].T.astype(np.float32), DC),
                "wqT": _strip((Wq[sl, :] * 0.125).T.astype(np.float32), DC),
                "wkT": _strip(Wk[sl, :].T.astype(np.float32), DC),
                "wvT": _strip(Wv[sl, :].T.astype(np.float32), DC),
                "woT": _strip(Wo[:, sl].T.astype(np.float32), 2),
                "tri": tri,
            }
        )
    return in_maps


def kernel(x, Wq, Wk, Wv, Wo, bo, _run_kwargs=None):
    x, Wq, Wk, Wv, Wo, bo = (
        np.asarray(a, dtype=np.float32) for a in (x, Wq, Wk, Wv, Wo, bo)
    )
    nc = build_nc()
    in_maps = make_in_maps(x, Wq, Wk, Wv, Wo)
    res = run_bass_kernel_spmd(
        nc, in_maps, core_ids=list(range(8)), **(_run_kwargs or {})
    )
    out = np.zeros((2, S, DD), dtype=np.float32)
    for c in range(8):
        out[c // 4] += res.results[c]["out"]
    out += bo[None, None, :]
    if _run_kwargs:
        kernel.last_results = res
    return out



# revision 15
# speedup vs baseline: 1.0367x; 1.0367x over previous
"""Multi-head causal attention (B=2, S=2048, D=1024, H=16) on 8 TRN2 NeuronCores.

Sharding: core c -> batch c//4, head-quarter c%4 (4 heads = 256 head dims).
Each core runs the full pipeline for its (batch, 4 heads); host sums the 4
row-sharded out-projection partials per batch + bias.

v3 design (fp8 DoubleRow projections, bf16 attention core):
  - QKV and out projections run as fp8e4m3 DoubleRow matmuls (0.5 PE
    cycles per output column, 256-deep contraction per instruction).
    Accuracy is preserved with a hi/lo residual split: a ~= Q(s*a) + Q(s*a
    - hi) with s chosen to keep residuals out of the fp8 subnormal floor
    (x: s=8, weights: s=32). Three products (hh, lh, hl) recover
    better-than-bf16 accuracy at 0.75x the bf16 PE cost for QKV and 0.75x
    for the out projection; descale 1/256 is folded into the existing
    PSUM->SBUF copies.
  - Scores: per 128-k-tile, live-q-trimmed moving qT (free 512-coff),
    bf16 (fp8 q/k measurably fails the 2e-2 gate).
  - exp on Act into bf16 `at` tiles; causal diag masked by tri-mult on Pool.
  - PV in stationary-attention layout: out[q,65] += at[k,q].T @ [v|1][k,65];
    all 4 heads accumulate in ONE PSUM bank (DVE memset pre-zeros the bank).
    The vS copy descale is 8/256 so PV directly yields 8*ctx (the fp8 hi/lo
    split scale) while the ones-column rowsum stays at true scale.
  - normalize: DVE reciprocal + one broadcast multiply into bf16 t; XBAR
    DMA-transpose of t gives d-major bf16 ctx, which Pool splits into fp8
    hi (copy) + lo (subtract). The out-projection DoubleRow pairs the two
    128-d halves per product (the ldweights AP pattern walrus accepts:
    contiguous columns, large pair stride).
Emission is software-pipelined by a FIFO fill queue paced to the Act
engine's exp rate: projection groups, PV blocks and out-proj tiles drain
into the scores i-loop's Act-bound surplus, with out-proj units for the
first 12 row-tiles hoarded for chunk 3's long exp-only stretch.
"""

import sys

import numpy as np

if "/opt/trn_rl_repo" not in sys.path:
    sys.path.insert(0, "/opt/trn_rl_repo")

import concourse.bass as bass
import concourse.mybir as mybir
import concourse.tile as tile
from concourse.bass import ts
from concourse.bass_utils import run_bass_kernel_spmd

P = 128          # partitions
S = 2048         # sequence length
DD = 1024        # model dim
DC = DD // P     # d-model chunks (8)
E = 256          # head dims per core (4 heads x 64)
H4 = 4           # heads per core
HD = 64
NQ = 4           # q chunks of 512
QC = 512
KT = S // P      # k tiles (16)
FD = 512         # out-proj free dim
DESCALE = 1.0 / 256.0   # 1/(8*32): undo the fp8 hi/lo split scaling

F32 = mybir.dt.float32
BF16 = mybir.dt.bfloat16
F8 = mybir.dt.float8e4
U16 = mybir.dt.uint16
DR = mybir.MatmulPerfMode.DoubleRow
EXP = mybir.ActivationFunctionType.Exp
MUL = mybir.AluOpType.mult
SUB = mybir.AluOpType.subtract

# (x-part, w-part) hi/lo product pairs: hh + lh + hl ~ full precision
PRODUCTS = ((0, 0), (1, 0), (0, 1))


def _emit(tc, nc, xT_d, wq_d, wk_d, wv_d, wo_d, tri_d, out_d):
    with (
        tc.tile_pool(name="const", bufs=1) as const,
        tc.tile_pool(name="attn", bufs=52) as attn_pool,
        tc.tile_pool(name="small", bufs=4) as small,
        tc.tile_pool(name="ctxp", bufs=4) as ctxp,
        tc.tile_pool(name="ostage", bufs=3) as ostage,
        tc.tile_pool(name="pmm", bufs=2, space="PSUM") as pmm,
        tc.tile_pool(name="pacc", bufs=2, space="PSUM") as pacc,
        tc.tile_pool(name="psc", bufs=2, space="PSUM") as psc,
    ):
        xT = const.tile([P, 2, DC, S], F8)
        wq = const.tile([P, 2, DC, E], F8)
        wk = const.tile([P, 2, DC, E], F8)
        wv = const.tile([P, 2, DC, E], F8)
        wo = const.tile([P, 2, 2, DD], F8)
        tri = const.tile([P, P], BF16)
        qT = const.tile([P, 2, S], BF16)
        kT = const.tile([P, 2, S], BF16)
        vS = const.tile([P, KT, H4, HD + 1], BF16)
        # ctx^T in fp8 hi/lo: [hi|lo, 128-d half, seq]; pair dim for the
        # out-proj DoubleRow is the 128-d half (proven ldweights pattern)
        cT = const.tile([P, 2, 2, S], F8)

        # j=0 slices of x^T first so the first projections can start early;
        # issue across both HWDGE queues (SP + Act) to halve serialization
        nc.scalar.dma_start(xT[:, 0, :, 0:QC], xT_d[:, 0, :, 0:QC])
        nc.sync.dma_start(wq[:], wq_d[:])
        nc.sync.dma_start(xT[:, 1, :, 0:QC], xT_d[:, 1, :, 0:QC])
        nc.scalar.dma_start(wk[:], wk_d[:])
        nc.scalar.dma_start(xT[:, 0, :, QC:S], xT_d[:, 0, :, QC:S])
        nc.sync.dma_start(wv[:], wv_d[:])
        nc.sync.dma_start(tri[:], tri_d[:])
        nc.sync.dma_start(xT[:, 1, :, QC:S], xT_d[:, 1, :, QC:S])
        nc.scalar.dma_start(wo[:], wo_d[:])

        # ones column of [V|1] (rowsums of masked exp-scores come out of PV)
        nc.vector.memset(vS[:, :, :, HD], 1.0)

        def psum_copy(dst, src):
            # all PSUM->SBUF copies on VectorE with the fp8-split descale
            nc.vector.tensor_scalar(dst, src, DESCALE, None, MUL)

        def emit_qk_proj(j):
            for w_s, dst in ((wq, qT), (wk, kT)):
                for et in range(2):
                    ps = pmm.tile([P, QC], F32, tag="mm", name="ps_proj")
                    first = True
                    for hx, hw in PRODUCTS:
                        for c2 in range(DC // 2):
                            nc.tensor.matmul(
                                ps[:],
                                lhsT=w_s[:, hw, 2 * c2:2 * c2 + 2, ts(et, P)],
                                rhs=xT[:, hx, 2 * c2:2 * c2 + 2, ts(j, QC)],
                                start=first,
                                stop=(hx == 0 and hw == 1 and c2 == DC // 2 - 1),
                                perf_mode=DR,
                            )
                            first = False
                    psum_copy(dst[:, et, ts(j, QC)], ps[:])

        def emit_v_proj(nt):
            psv = pmm.tile([P, E], F32, tag="mm", name="ps_v")
            first = True
            for hx, hw in PRODUCTS:
                for c2 in range(DC // 2):
                    nc.tensor.matmul(
                        psv[:],
                        lhsT=xT[:, hx, 2 * c2:2 * c2 + 2, ts(nt, P)],
                        rhs=wv[:, hw, 2 * c2:2 * c2 + 2, :],
                        start=first,
                        stop=(hx == 0 and hw == 1 and c2 == DC // 2 - 1),
                        perf_mode=DR,
                    )
                    first = False
            # descale 8/256: vS holds v/4 so PV yields 8*ctx (fp8 split scale)
            nc.vector.tensor_scalar(
                vS[:, nt, :, 0:HD],
                psv[:].rearrange("p (h d) -> p h d", h=H4),
                8.0 * DESCALE, None, MUL,
            )

        def emit_scores(j, i, at_tiles):
            # scores + exp for k-tile i against q-chunk j, both head pairs
            coff = max(0, P * (i - 4 * j))
            for hp in range(2):
                sc = psc.tile([P, 2, QC], F32, tag="sc", name="sc")
                at = attn_pool.tile([P, 2, QC], BF16, tag="at", name="at")
                for hh in range(2):
                    po = HD * hh
                    nc.tensor.matmul(
                        sc[:, hh, coff:QC],
                        lhsT=kT[po:po + HD, hp, ts(i, P)],
                        rhs=qT[po:po + HD, hp, j * QC + coff:(j + 1) * QC],
                        start=True,
                        stop=True,
                    )
                nc.scalar.activation(at[:, :, coff:QC], sc[:, :, coff:QC], EXP)
                if i >= 4 * j:  # diagonal 128x128 block: causal triangle
                    for hh in range(2):
                        nc.gpsimd.tensor_tensor(
                            at[:, hh, coff:coff + P],
                            at[:, hh, coff:coff + P],
                            tri[:],
                            MUL,
                        )
                at_tiles[i, hp] = at

        def emit_pv(j, qs, at_tiles):
            # ctx[q, d] for global q-subtile qs, all 4 heads in one PSUM bank.
            qo = P * (qs - 4 * j)
            # explicit zero-init (hardware start_tensor_calc only overwrites
            # bytes each matmul writes, so the four head-groups sharing one
            # bank must accumulate start=False onto real zeros). Tiles are
            # pre-staged so the next PV's memset sits ahead of this PV's
            # normalize ops in the DVE queue.
            if pacc_pre:
                pv = pacc_pre.pop()
            else:
                pv = pacc.tile([P, H4, HD + 1], F32, tag="pv", name="pv")
                nc.vector.memset(pv[:], 0.0)

            for hp in range(2):
                for hh in range(2):
                    h = 2 * hp + hh
                    for i in range(qs + 1):
                        nc.tensor.matmul(
                            pv[:, h, :],
                            lhsT=at_tiles[i, hp][:, hh, qo:qo + P],
                            rhs=vS[:, i, h, :],
                            start=False,
                            stop=(hh == 1 and i == qs),
                            skip_group_check=True,
                        )
            prestage_pacc()

            # normalize: t = (8*ctx_unnorm) / rowsum (PV already carries the
            # x8 split scale); XBAR-transpose t to d-major bf16, then Pool
            # splits into fp8 hi + lo for the DoubleRow out-projection.
            rec = small.tile([P, H4], F32, tag="rec", name="rec")
            t = small.tile([P, H4, HD], BF16, tag="t", name="t")
            cb = ctxp.tile([P, 2, P], BF16, tag="cb", name="cb")
            nc.vector.reciprocal(rec[:], pv[:, :, HD])
            nc.vector.tensor_tensor(
                t[:], pv[:, :, 0:HD],
                rec[:].unsqueeze(2).broadcast_to([P, H4, HD]), MUL,
            )
            for hp in range(2):
                nc.sync.dma_start(
                    out=cb[:, hp, :],
                    in_=t[:, 2 * hp:2 * hp + 2, :],
                    transpose=True,
                )
            nc.gpsimd.tensor_copy(cT[:, 0, :, ts(qs, P)], cb[:])
            nc.gpsimd.tensor_tensor(
                cT[:, 1, :, ts(qs, P)], cb[:], cT[:, 0, :, ts(qs, P)], SUB,
            )

        ob_half = {}
        pacc_pre = []
        pv_left = [4 * KT // 4]  # 16 PV subtiles total

        def prestage_pacc():
            pv_left[0] -= 1
            if pv_left[0] > 0 and not pacc_pre:
                t = pacc.tile([P, H4, HD + 1], F32, tag="pv", name="pv")
                nc.vector.memset(t[:], 0.0)
                pacc_pre.append(t)

        def emit_out_proj(nt, fc, tail=False):
            po = pmm.tile([P, FD], F32, tag="mm", name="ps_out")
            first = True
            for hc, hw in PRODUCTS:
                nc.tensor.matmul(
                    po[:],
                    lhsT=cT[:, hc, :, ts(nt, P)],
                    rhs=wo[:, hw, :, ts(fc, FD)],
                    start=first,
                    stop=(hc == 0 and hw == 1),
                    perf_mode=DR,
                )
                first = False
            if tail:
                # tail latency: separate half-DMAs, copies on idle Act,
                # issues alternating between the two HWDGE queues
                ob = ostage.tile([P, FD], F32, tag="obt", name="obt")
                if fc == 0:  # parallel tail copies on Act and DVE
                    nc.scalar.activation(
                        ob[:], po[:],
                        mybir.ActivationFunctionType.Copy, scale=DESCALE,
                    )
                    nc.sync.dma_start(out_d[ts(nt, P), ts(fc, FD)], ob[:])
                else:
                    psum_copy(ob[:], po[:])
                    nc.scalar.dma_start(out_d[ts(nt, P), ts(fc, FD)], ob[:])
                return
            # merge both halves of a row-tile into one staging tile and a
            # single [128, 1024] DMA: halves the SP issue cost
            if fc == 0:
                ob_half[nt] = ostage.tile([P, 2, FD], F32, tag="ob", name="ob")
            ob = ob_half[nt]
            if nt >= 13 and fc == 1:  # Act has slack in the wind-down
                nc.scalar.activation(
                    ob[:, fc, :], po[:],
                    mybir.ActivationFunctionType.Copy, scale=DESCALE,
                )
            else:
                psum_copy(ob[:, fc, :], po[:])
            if fc == 1:
                nc.sync.dma_start(out_d[ts(nt, P), :], ob_half.pop(nt)[:])

        # Global software pipeline. The scores i-loop is Act(exp)-bound, so
        # every other PE unit (projection groups, PV blocks, out-proj tiles)
        # goes into a FIFO fill queue drained per-iteration by the Act-pace
        # surplus. Un-drained units carry across chunk boundaries, which
        # automatically defers PV/out-proj work into chunk 3's long
        # exp-only stretch.
        at_tiles = {}
        fill = []  # (cost_ns, kind, emit_fn)

        pace = [0.0]  # cumulative Act-surplus vs drained PE cost

        def drain(budget):
            # surplus while the queue is empty is wasted, not banked
            pace[0] = min(pace[0] + budget, 4000.0)
            while fill and pace[0] > 0:
                cost, _, fn = fill.pop(0)
                fn()
                pace[0] -= cost

        def force_drain(kind):
            # emit every queued unit up to and including the last of `kind`
            last = max((n for n, (_, k, _) in enumerate(fill) if k == kind),
                       default=-1)
            for _ in range(last + 1):
                _, _, fn = fill.pop(0)
                fn()

        deferred_out = []  # out-proj units hoarded for chunk 3's exp stretch
        in_last = [False]

        def push_pv(j, qs):
            cost = (qs + 1) * H4 * (HD + 1) * 0.42 + 600
            snap = dict(at_tiles)  # at_tiles mutates before deferred drain
            def go():
                emit_pv(j, qs, snap)
                dst = fill if (in_last[0] or qs >= 12) else deferred_out
                tail = qs >= 15
                dst.append((430, "out", lambda: emit_out_proj(qs, 0, tail)))
                dst.append((430, "out", lambda: emit_out_proj(qs, 1, tail)))
            fill.append((cost, "pv", go))

        emit_qk_proj(0)
        for nt in range(4):
            emit_v_proj(nt)

        for j in range(NQ):
            if j + 1 < NQ:
                fill.append((5200, f"qk{j + 1}", lambda j1=j + 1: emit_qk_proj(j1)))
                for nt in range(4 * (j + 1), 4 * (j + 1) + 4):
                    fill.append((700, f"v{j + 1}", lambda nt=nt: emit_v_proj(nt)))
            if j > 0:
                force_drain(f"qk{j}")  # this chunk's q/k must exist
            if j == NQ - 1:
                in_last[0] = True
                fill.extend(deferred_out)
                deferred_out.clear()
            nk = 4 * (j + 1)
            for i in range(nk):
                if i == 4 * j and j > 0:
                    force_drain(f"v{j}")  # PV(j, 4j) is imminent
                emit_scores(j, i, at_tiles)
                # Act per iter: 2 exps of 2*(QC-coff) free each + overhead;
                # scores PE cost: 4 matmuls of (QC-coff) cols. Drain just
                # under the surplus so Act never waits on the next scores.
                w = QC - max(0, P * (i - 4 * j))
                drain(1.46 * w + 400)
                if i > 4 * j:
                    push_pv(j, i - 1)
            push_pv(j, 4 * j + 3)
        # final flush: the last PV goes first so its normalize/transpose
        # chain drains while PE chews the remaining out-proj tiles
        fill.sort(key=lambda u: 0 if u[1] == "pv" else 1)
        while fill:
            _, _, fn = fill.pop(0)
            fn()


def _split_multi_waits(nc):
    """The TRN2 instruction encoding carries ONE sync-wait slot; this walrus
    build rejects instructions with more. Hoist extra waits onto standalone
    EventSemaphore instructions immediately before (same engine queue, same
    semantics)."""
    n = 0
    for f in nc.m.functions:
        for b in f.blocks:
            out = []
            for i in list(b.instructions):
                si = i.sync_info
                if si is not None and len(si.on_wait) > 1:
                    waits = list(si.on_wait)
                    for w in waits[:-1]:
                        n += 1
                        out.append(
                            mybir.InstEventSemaphore(
                                name=f"I-wsplit{n}",
                                engine=i.engine,
                                ins=[],
                                outs=[],
                                sync_info=mybir.SyncInfo(on_wait=[w], on_update=[]),
                            )
                        )
                    i.sync_info = mybir.SyncInfo(
                        on_wait=[waits[-1]], on_update=list(si.on_update)
                    )
                out.append(i)
            b.instructions = out


def build_nc(split_waits=True):
    nc = bass.Bass("TRN2", target_bir_lowering=False, debug=False)
    xT_d = nc.dram_tensor("xT", [P, 2, DC, S], F8, kind="ExternalInput").ap()
    wq_d = nc.dram_tensor("wqT", [P, 2, DC, E], F8, kind="ExternalInput").ap()
    wk_d = nc.dram_tensor("wkT", [P, 2, DC, E], F8, kind="ExternalInput").ap()
    wv_d = nc.dram_tensor("wvT", [P, 2, DC, E], F8, kind="ExternalInput").ap()
    wo_d = nc.dram_tensor("woT", [P, 2, 2, DD], F8, kind="ExternalInput").ap()
    tri_d = nc.dram_tensor("tri", [P, P], BF16, kind="ExternalInput").ap()
    out_d = nc.dram_tensor("out", [S, DD], F32, kind="ExternalOutput").ap()
    with tile.TileContext(nc) as tc:
        _emit(tc, nc, xT_d, wq_d, wk_d, wv_d, wo_d, tri_d, out_d)
    if split_waits:
        _split_multi_waits(nc)
    return nc


def _hilo(a):
    """fp32 array -> stacked (hi, lo) fp8e4m3 split along a new axis 0."""
    import ml_dtypes

    f8 = ml_dtypes.float8_e4m3
    hi = a.astype(f8)
    lo = (a - hi.astype(np.float32)).astype(f8)
    return np.ascontiguousarray(np.stack([hi, lo], axis=0))


def _strip(a, chunks):
    """[D, N] -> [D//128 chunks, 128, N] -> [128, chunks, N] fp32."""
    d, n = a.shape
    return a.reshape(chunks, P, n).transpose(1, 0, 2)


def make_in_maps(x, Wq, Wk, Wv, Wo):
    import ml_dtypes

    tri = np.ascontiguousarray(
        np.triu(np.ones((P, P), np.float32)).astype(ml_dtypes.bfloat16)
    )
    in_maps = []
    for c in range(8):
        b, g = c // 4, c % 4
        sl = slice(E * g, E * (g + 1))
        # x scaled x8, weights x32 (kept out of the fp8 subnormal floor);
        # kernel folds the 1/256 descale into its PSUM->SBUF copies.
        xs = _hilo(_strip(8.0 * x[b].T.astype(np.float32), DC))
        wqs = _hilo(_strip((Wq[sl, :] * (0.125 * 32)).T.astype(np.float32), DC))
        wks = _hilo(_strip((Wk[sl, :] * 32).T.astype(np.float32), DC))
        wvs = _hilo(_strip((Wv[sl, :] * 32).T.astype(np.float32), DC))
        # wo: [256 d, 1024] -> [128, 2 d-chunks, 1024] chunk-major, scaled x32
        wos = _hilo(_strip((Wo[:, sl] * 32).T.astype(np.float32), 2))
        in_maps.append(
            {
                "xT": np.ascontiguousarray(xs.transpose(1, 0, 2, 3)),
                "wqT": np.ascontiguousarray(wqs.transpose(1, 0, 2, 3)),
                "wkT": np.ascontiguousarray(wks.transpose(1, 0, 2, 3)),
                "wvT": np.ascontiguousarray(wvs.transpose(1, 0, 2, 3)),
                "woT": np.ascontiguousarray(wos.transpose(1, 0, 2, 3)),
                "tri": tri,
            }
        )
    return in_maps


def kernel(x, Wq, Wk, Wv, Wo, bo, _run_kwargs=None):
    x, Wq, Wk, Wv, Wo, bo = (
        np.asarray(a, dtype=np.float32) for a in (x, Wq, Wk, Wv, Wo, bo)
    )
    nc = build_nc()
    in_maps = make_in_maps(x, Wq, Wk, Wv, Wo)
    res = run_bass_kernel_spmd(
        nc, in_maps, core_ids=list(range(8)), **(_run_kwargs or {})
    )
    out = np.zeros((2, S, DD), dtype=np.float32)
    for c in range(8):
        out[c // 4] += res.results[c]["out"]
    out += bo[None, None, :]
    if _run_kwargs:
        kernel.last_results = res
    return out


# revision 23
# speedup vs baseline: 1.0602x; 1.0227x over previous
"""Multi-head causal attention (B=2, S=2048, D=1024, H=16) on 8 TRN2 NeuronCores.

Sharding: core c -> batch c//4, head-quarter c%4 (4 heads = 256 head dims).
Each core runs the full pipeline for its (batch, 4 heads); host sums the 4
row-sharded out-projection partials per batch + bias.

v3 design (fp8 DoubleRow projections, bf16 attention core):
  - QKV and out projections run as fp8e4m3 DoubleRow matmuls (0.5 PE
    cycles per output column, 256-deep contraction per instruction).
    Accuracy is preserved with a hi/lo residual split: a ~= Q(s*a) + Q(s*a
    - hi) with s chosen to keep residuals out of the fp8 subnormal floor
    (x: s=8, weights: s=32). Three products (hh, lh, hl) recover
    better-than-bf16 accuracy at 0.75x the bf16 PE cost for QKV and 0.75x
    for the out projection; descale 1/256 is folded into the existing
    PSUM->SBUF copies.
  - Scores: per 128-k-tile, live-q-trimmed moving qT (free 512-coff),
    bf16 (fp8 q/k measurably fails the 2e-2 gate).
  - exp on Act into bf16 `at` tiles; causal diag masked by tri-mult on Pool.
  - PV in stationary-attention layout: out[q,65] += at[k,q].T @ [v|1][k,65];
    all 4 heads accumulate in ONE PSUM bank (DVE memset pre-zeros the bank).
    The vS copy descale is 8/256 so PV directly yields 8*ctx (the fp8 hi/lo
    split scale) while the ones-column rowsum stays at true scale.
  - normalize: DVE reciprocal + one broadcast multiply into bf16 t; XBAR
    DMA-transpose of t gives d-major bf16 ctx, which Pool splits into fp8
    hi (copy) + lo (subtract). The out-projection DoubleRow pairs the two
    128-d halves per product (the ldweights AP pattern walrus accepts:
    contiguous columns, large pair stride).
Emission is software-pipelined by a FIFO fill queue paced to the Act
engine's exp rate: projection groups, PV blocks and out-proj tiles drain
into the scores i-loop's Act-bound surplus, with out-proj units for the
first 12 row-tiles hoarded for chunk 3's long exp-only stretch.
"""

import sys

import numpy as np

if "/opt/trn_rl_repo" not in sys.path:
    sys.path.insert(0, "/opt/trn_rl_repo")

import concourse.bass as bass
import concourse.mybir as mybir
import concourse.tile as tile
from concourse.bass import ts
from concourse.bass_utils import run_bass_kernel_spmd

P = 128          # partitions
S = 2048         # sequence length
DD = 1024        # model dim
DC = DD // P     # d-model chunks (8)
E = 256          # head dims per core (4 heads x 64)
H4 = 4           # heads per core
HD = 64
NQ = 4           # q chunks of 512
QC = 512
KT = S // P      # k tiles (16)
FD = 512         # out-proj free dim
DESCALE = 1.0 / 256.0   # 1/(8*32): undo the fp8 hi/lo split scaling

F32 = mybir.dt.float32
BF16 = mybir.dt.bfloat16
F8 = mybir.dt.float8e4
U16 = mybir.dt.uint16
DR = mybir.MatmulPerfMode.DoubleRow
EXP = mybir.ActivationFunctionType.Exp
MUL = mybir.AluOpType.mult
SUB = mybir.AluOpType.subtract

# (x-part, w-part) hi/lo product pairs: hh + lh + hl ~ full precision
PRODUCTS = ((0, 0), (1, 0), (0, 1))


def _emit(tc, nc, xT_d, wq_d, wk_d, wv_d, wo_d, tri_d, out_d):
    with (
        tc.tile_pool(name="const", bufs=1) as const,
        tc.tile_pool(name="attn", bufs=52) as attn_pool,
        tc.tile_pool(name="small", bufs=4) as small,
        tc.tile_pool(name="ctxp", bufs=4) as ctxp,
        tc.tile_pool(name="ostage", bufs=3) as ostage,
        tc.tile_pool(name="pmm", bufs=2, space="PSUM") as pmm,
        tc.tile_pool(name="pacc", bufs=2, space="PSUM") as pacc,
        tc.tile_pool(name="psc", bufs=2, space="PSUM") as psc,
    ):
        xT = const.tile([P, 2, DC, S], F8)
        wq = const.tile([P, 2, DC, E], F8)
        wk = const.tile([P, 2, DC, E], F8)
        wv = const.tile([P, 2, DC, E], F8)
        wo = const.tile([P, 2, 2, DD], F8)
        tri = const.tile([P, P], BF16)
        qT = const.tile([P, 2, S], BF16)
        kT = const.tile([P, 2, S], BF16)
        vS = const.tile([P, KT, H4, HD + 1], BF16)
        # ctx^T in fp8 hi/lo: [hi|lo, 128-d half, seq]; pair dim for the
        # out-proj DoubleRow is the 128-d half (proven ldweights pattern)
        cT = const.tile([P, 2, 2, S], F8)

        # j=0 slices of x^T and the q/k weights first so the first projection
        # products can start early; issue across both HWDGE queues (SP + Act)
        # to halve serialization. hi parts lead (first product is hi*hi).
        nc.scalar.dma_start(xT[:, 0, :, 0:QC], xT_d[:, 0, :, 0:QC])
        nc.sync.dma_start(wq[:], wq_d[:])
        nc.sync.dma_start(xT[:, 1, :, 0:QC], xT_d[:, 1, :, 0:QC])
        nc.scalar.dma_start(wk[:], wk_d[:])
        nc.scalar.dma_start(xT[:, 0, :, QC:S], xT_d[:, 0, :, QC:S])
        nc.sync.dma_start(wv[:], wv_d[:])
        nc.sync.dma_start(tri[:], tri_d[:])
        nc.sync.dma_start(xT[:, 1, :, QC:S], xT_d[:, 1, :, QC:S])
        nc.scalar.dma_start(wo[:], wo_d[:])

        # ones column of [V|1] (rowsums of masked exp-scores come out of PV)
        nc.vector.memset(vS[:, :, :, HD], 1.0)

        def psum_copy(dst, src):
            # all PSUM->SBUF copies on VectorE with the fp8-split descale
            nc.vector.tensor_scalar(dst, src, DESCALE, None, MUL)

        def emit_qk_group(w_s, dst, et, j, lo=0, hi=QC):
            # fp8 DoubleRow: each matmul contracts a PAIR of 128-d chunks
            # at 0.5 cycles per output column; 3 hi/lo products accumulate.
            ps = pmm.tile([P, hi - lo], F32, tag="mm", name="ps_proj")
            first = True
            for hx, hw in PRODUCTS:
                for c2 in range(DC // 2):
                    nc.tensor.matmul(
                        ps[:],
                        lhsT=w_s[:, hw, 2 * c2:2 * c2 + 2, ts(et, P)],
                        rhs=xT[:, hx, 2 * c2:2 * c2 + 2, j * QC + lo:j * QC + hi],
                        start=first,
                        stop=(hx == 0 and hw == 1 and c2 == DC // 2 - 1),
                        perf_mode=DR,
                    )
                    first = False
            psum_copy(dst[:, et, j * QC + lo:j * QC + hi], ps[:])

        def emit_v_proj(nt):
            psv = pmm.tile([P, E], F32, tag="mm", name="ps_v")
            first = True
            for hx, hw in PRODUCTS:
                for c2 in range(DC // 2):
                    nc.tensor.matmul(
                        psv[:],
                        lhsT=xT[:, hx, 2 * c2:2 * c2 + 2, ts(nt, P)],
                        rhs=wv[:, hw, 2 * c2:2 * c2 + 2, :],
                        start=first,
                        stop=(hx == 0 and hw == 1 and c2 == DC // 2 - 1),
                        perf_mode=DR,
                    )
                    first = False
            # descale 8/256: vS holds v/4 so PV yields 8*ctx (fp8 split scale)
            nc.vector.tensor_scalar(
                vS[:, nt, :, 0:HD],
                psv[:].rearrange("p (h d) -> p h d", h=H4),
                8.0 * DESCALE, None, MUL,
            )

        def emit_scores(j, i, at_tiles):
            # scores + exp for k-tile i against q-chunk j, both head pairs
            coff = max(0, P * (i - 4 * j))
            for hp in range(2):
                sc = psc.tile([P, 2, QC], F32, tag="sc", name="sc")
                at = attn_pool.tile([P, 2, QC], BF16, tag="at", name="at")
                for hh in range(2):
                    po = HD * hh
                    nc.tensor.matmul(
                        sc[:, hh, coff:QC],
                        lhsT=kT[po:po + HD, hp, ts(i, P)],
                        rhs=qT[po:po + HD, hp, j * QC + coff:(j + 1) * QC],
                        start=True,
                        stop=True,
                    )
                nc.scalar.activation(at[:, :, coff:QC], sc[:, :, coff:QC], EXP)
                if i >= 4 * j:  # diagonal 128x128 block: causal triangle
                    for hh in range(2):
                        nc.gpsimd.tensor_tensor(
                            at[:, hh, coff:coff + P],
                            at[:, hh, coff:coff + P],
                            tri[:],
                            MUL,
                        )
                at_tiles[i, hp] = at

        def emit_pv(j, qs, at_tiles, split=False):
            # ctx[q, d] for global q-subtile qs, all 4 heads in one PSUM bank.
            qo = P * (qs - 4 * j)
            # explicit zero-init (hardware start_tensor_calc only overwrites
            # bytes each matmul writes, so the four head-groups sharing one
            # bank must accumulate start=False onto real zeros). Tiles are
            # pre-staged so the next PV's memset sits ahead of this PV's
            # normalize ops in the DVE queue.
            if pacc_pre:
                pv = pacc_pre.pop()
            else:
                pv = pacc.tile([P, H4, HD + 1], F32, tag="pv", name="pv")
                nc.vector.memset(pv[:], 0.0)

            # normalize: t = (8*ctx_unnorm) / rowsum (PV already carries the
            # x8 split scale); XBAR-transpose t to d-major bf16, then Pool
            # splits into fp8 hi + lo for the DoubleRow out-projection.
            # split=True (final subtile) normalizes per head-pair so the
            # hp0 chain drains while PE runs hp1's matmuls: shorter tail.
            cb = ctxp.tile([P, 2, P], BF16, tag="cb", name="cb")

            def normalize(lo, hi):
                rec = small.tile([P, hi - lo], F32, tag="rec", name="rec")
                t = small.tile([P, hi - lo, HD], BF16, tag="t", name="t")
                nc.vector.reciprocal(rec[:], pv[:, lo:hi, HD])
                nc.vector.tensor_tensor(
                    t[:], pv[:, lo:hi, 0:HD],
                    rec[:].unsqueeze(2).broadcast_to([P, hi - lo, HD]), MUL,
                )
                for hp in range(lo // 2, hi // 2):
                    nc.sync.dma_start(
                        out=cb[:, hp, :],
                        in_=t[:, 2 * hp - lo:2 * hp + 2 - lo, :],
                        transpose=True,
                    )

            for hp in range(2):
                for hh in range(2):
                    h = 2 * hp + hh
                    for i in range(qs + 1):
                        nc.tensor.matmul(
                            pv[:, h, :],
                            lhsT=at_tiles[i, hp][:, hh, qo:qo + P],
                            rhs=vS[:, i, h, :],
                            start=False,
                            stop=(hh == 1 and i == qs),
                            skip_group_check=True,
                        )
                if split:
                    normalize(2 * hp, 2 * hp + 2)
            prestage_pacc()
            if not split:
                normalize(0, H4)
            nc.gpsimd.tensor_copy(cT[:, 0, :, ts(qs, P)], cb[:])
            nc.gpsimd.tensor_tensor(
                cT[:, 1, :, ts(qs, P)], cb[:], cT[:, 0, :, ts(qs, P)], SUB,
            )

        ob_half = {}
        pacc_pre = []
        pv_left = [4 * KT // 4]  # 16 PV subtiles total

        def prestage_pacc():
            pv_left[0] -= 1
            if pv_left[0] > 0 and not pacc_pre:
                t = pacc.tile([P, H4, HD + 1], F32, tag="pv", name="pv")
                nc.vector.memset(t[:], 0.0)
                pacc_pre.append(t)

        def emit_out_proj(nt, fc, tail=False):
            po = pmm.tile([P, FD], F32, tag="mm", name="ps_out")
            first = True
            for hc, hw in PRODUCTS:
                nc.tensor.matmul(
                    po[:],
                    lhsT=cT[:, hc, :, ts(nt, P)],
                    rhs=wo[:, hw, :, ts(fc, FD)],
                    start=first,
                    stop=(hc == 0 and hw == 1),
                    perf_mode=DR,
                )
                first = False
            if tail:
                # tail latency: separate half-DMAs, copies on idle Act,
                # issues alternating between the two HWDGE queues
                ob = ostage.tile([P, FD], F32, tag="obt", name="obt")
                if fc == 0:  # parallel tail copies on Act and DVE
                    nc.scalar.activation(
                        ob[:], po[:],
                        mybir.ActivationFunctionType.Copy, scale=DESCALE,
                    )
                    nc.sync.dma_start(out_d[ts(nt, P), ts(fc, FD)], ob[:])
                else:
                    psum_copy(ob[:], po[:])
                    nc.scalar.dma_start(out_d[ts(nt, P), ts(fc, FD)], ob[:])
                return
            # merge both halves of a row-tile into one staging tile and a
            # single [128, 1024] DMA: halves the SP issue cost
            if fc == 0:
                ob_half[nt] = ostage.tile([P, 2, FD], F32, tag="ob", name="ob")
            ob = ob_half[nt]
            if nt >= 13 and fc == 1:  # Act has slack in the wind-down
                nc.scalar.activation(
                    ob[:, fc, :], po[:],
                    mybir.ActivationFunctionType.Copy, scale=DESCALE,
                )
            else:
                psum_copy(ob[:, fc, :], po[:])
            if fc == 1:
                nc.sync.dma_start(out_d[ts(nt, P), :], ob_half.pop(nt)[:])

        # Global software pipeline. The scores i-loop is Act(exp)-bound, so
        # every other PE unit (projection groups, PV blocks, out-proj tiles)
        # goes into a FIFO fill queue drained per-iteration by the Act-pace
        # surplus. Un-drained units carry across chunk boundaries, which
        # automatically defers PV/out-proj work into chunk 3's long
        # exp-only stretch.
        at_tiles = {}
        fill = []  # (cost_ns, kind, emit_fn)

        pace = [0.0]  # cumulative Act-surplus vs drained PE cost

        def drain(budget):
            # surplus while the queue is empty is wasted, not banked
            pace[0] = min(pace[0] + budget, 4000.0)
            while fill and pace[0] > 0:
                cost, _, fn = fill.pop(0)
                fn()
                pace[0] -= cost

        def force_drain(kind):
            # emit every queued unit up to and including the last of `kind`
            last = max((n for n, (_, k, _) in enumerate(fill) if k == kind),
                       default=-1)
            for _ in range(last + 1):
                _, _, fn = fill.pop(0)
                fn()

        deferred_out = []  # out-proj units hoarded for chunk 3's exp stretch
        in_last = [False]

        def push_pv(j, qs):
            cost = (qs + 1) * H4 * (HD + 1) * 0.42 + 600
            snap = dict(at_tiles)  # at_tiles mutates before deferred drain
            def go():
                emit_pv(j, qs, snap, split=(qs >= 15))
                dst = fill if (in_last[0] or qs >= 12) else deferred_out
                tail = qs >= 15
                dst.append((430, "out", lambda: emit_out_proj(qs, 0, tail)))
                dst.append((430, "out", lambda: emit_out_proj(qs, 1, tail)))
            fill.append((cost, "pv", go))

        # startup: full q chunks first (their DMAs land first), then tiny
        # k-tile-0 blocks, so the first scores+exp fire as early as
        # possible. The rest of j=0's k columns and all v tiles go through
        # the fill queue.
        for et in range(2):
            emit_qk_group(wq, qT, et, 0)
        for et in range(2):
            emit_qk_group(wk, kT, et, 0, 0, P)
        for kb in range(1, 4):
            for et in range(2):
                fill.append((560, f"kb{kb}",
                             lambda et=et, kb=kb: emit_qk_group(
                                 wk, kT, et, 0, kb * P, (kb + 1) * P)))
        for nt in range(4):
            fill.append((700, "v0", lambda nt=nt: emit_v_proj(nt)))

        for j in range(NQ):
            if j + 1 < NQ:
                for w_s, dst, nm in ((wq, qT, "q"), (wk, kT, "k")):
                    for et in range(2):
                        fill.append((1560, f"qk{j + 1}",
                                     lambda w_s=w_s, dst=dst, et=et, j1=j + 1:
                                     emit_qk_group(w_s, dst, et, j1)))
                for nt in range(4 * (j + 1), 4 * (j + 1) + 4):
                    fill.append((700, f"v{j + 1}", lambda nt=nt: emit_v_proj(nt)))
            if j > 0:
                force_drain(f"qk{j}")  # this chunk's q/k must exist
            if j == NQ - 1:
                in_last[0] = True
                fill.extend(deferred_out)
                deferred_out.clear()
            nk = 4 * (j + 1)
            for i in range(nk):
                if j == 0 and 1 <= i <= 3:
                    force_drain(f"kb{i}")  # k-tile i columns must exist
                if i == 4 * j and j > 0:
                    force_drain(f"v{j}")  # PV(j, 4j) is imminent
                emit_scores(j, i, at_tiles)
                # Act per iter: 2 exps of 2*(QC-coff) free each + overhead;
                # scores PE cost: 4 matmuls of (QC-coff) cols. Slightly
                # overfeed the PE (Act has global slack; PE is the
                # critical engine) so PE never stalls waiting for scores.
                w = QC - max(0, P * (i - 4 * j))
                drain(1.75 * w + 400)
                if i > 4 * j:
                    push_pv(j, i - 1)
            push_pv(j, 4 * j + 3)
        # final flush: the last PV goes first so its normalize/transpose
        # chain drains while PE chews the remaining out-proj tiles
        fill.sort(key=lambda u: 0 if u[1] == "pv" else 1)
        while fill:
            _, _, fn = fill.pop(0)
            fn()


def _split_multi_waits(nc):
    """The TRN2 instruction encoding carries ONE sync-wait slot; this walrus
    build rejects instructions with more. Hoist extra waits onto standalone
    EventSemaphore instructions immediately before (same engine queue, same
    semantics)."""
    n = 0
    for f in nc.m.functions:
        for b in f.blocks:
            out = []
            for i in list(b.instructions):
                si = i.sync_info
                if si is not None and len(si.on_wait) > 1:
                    waits = list(si.on_wait)
                    for w in waits[:-1]:
                        n += 1
                        out.append(
                            mybir.InstEventSemaphore(
                                name=f"I-wsplit{n}",
                                engine=i.engine,
                                ins=[],
                                outs=[],
                                sync_info=mybir.SyncInfo(on_wait=[w], on_update=[]),
                            )
                        )
                    i.sync_info = mybir.SyncInfo(
                        on_wait=[waits[-1]], on_update=list(si.on_update)
                    )
                out.append(i)
            b.instructions = out


def build_nc(split_waits=True):
    nc = bass.Bass("TRN2", target_bir_lowering=False, debug=False)
    xT_d = nc.dram_tensor("xT", [P, 2, DC, S], F8, kind="ExternalInput").ap()
    wq_d = nc.dram_tensor("wqT", [P, 2, DC, E], F8, kind="ExternalInput").ap()
    wk_d = nc.dram_tensor("wkT", [P, 2, DC, E], F8, kind="ExternalInput").ap()
    wv_d = nc.dram_tensor("wvT", [P, 2, DC, E], F8, kind="ExternalInput").ap()
    wo_d = nc.dram_tensor("woT", [P, 2, 2, DD], F8, kind="ExternalInput").ap()
    tri_d = nc.dram_tensor("tri", [P, P], BF16, kind="ExternalInput").ap()
    out_d = nc.dram_tensor("out", [S, DD], F32, kind="ExternalOutput").ap()
    with tile.TileContext(nc) as tc:
        _emit(tc, nc, xT_d, wq_d, wk_d, wv_d, wo_d, tri_d, out_d)
    if split_waits:
        _split_multi_waits(nc)
    return nc


def _hilo(a):
    """fp32 array -> stacked (hi, lo) fp8e4m3 split along a new axis 0."""
    import ml_dtypes

    f8 = ml_dtypes.float8_e4m3
    hi = a.astype(f8)
    lo = (a - hi.astype(np.float32)).astype(f8)
    return np.ascontiguousarray(np.stack([hi, lo], axis=0))


def _strip(a, chunks):
    """[D, N] -> [D//128 chunks, 128, N] -> [128, chunks, N] fp32."""
    d, n = a.shape
    return a.reshape(chunks, P, n).transpose(1, 0, 2)


def make_in_maps(x, Wq, Wk, Wv, Wo):
    import ml_dtypes

    tri = np.ascontiguousarray(
        np.triu(np.ones((P, P), np.float32)).astype(ml_dtypes.bfloat16)
    )
    in_maps = []
    for c in range(8):
        b, g = c // 4, c % 4
        sl = slice(E * g, E * (g + 1))
        # x scaled x8, weights x32 (kept out of the fp8 subnormal floor);
        # kernel folds the 1/256 descale into its PSUM->SBUF copies.
        xs = _hilo(_strip(8.0 * x[b].T.astype(np.float32), DC))
        wqs = _hilo(_strip((Wq[sl, :] * (0.125 * 32)).T.astype(np.float32), DC))
        wks = _hilo(_strip((Wk[sl, :] * 32).T.astype(np.float32), DC))
        wvs = _hilo(_strip((Wv[sl, :] * 32).T.astype(np.float32), DC))
        # wo: [256 d, 1024] -> [128, 2 d-chunks, 1024] chunk-major, scaled x32
        wos = _hilo(_strip((Wo[:, sl] * 32).T.astype(np.float32), 2))
        in_maps.append(
            {
                "xT": np.ascontiguousarray(xs.transpose(1, 0, 2, 3)),
                "wqT": np.ascontiguousarray(wqs.transpose(1, 0, 2, 3)),
                "wkT": np.ascontiguousarray(wks.transpose(1, 0, 2, 3)),
                "wvT": np.ascontiguousarray(wvs.transpose(1, 0, 2, 3)),
                "woT": np.ascontiguousarray(wos.transpose(1, 0, 2, 3)),
                "tri": tri,
            }
        )
    return in_maps


def kernel(x, Wq, Wk, Wv, Wo, bo, _run_kwargs=None):
    x, Wq, Wk, Wv, Wo, bo = (
        np.asarray(a, dtype=np.float32) for a in (x, Wq, Wk, Wv, Wo, bo)
    )
    nc = build_nc()
    in_maps = make_in_maps(x, Wq, Wk, Wv, Wo)
    res = run_bass_kernel_spmd(
        nc, in_maps, core_ids=list(range(8)), **(_run_kwargs or {})
    )
    out = np.zeros((2, S, DD), dtype=np.float32)
    for c in range(8):
        out[c // 4] += res.results[c]["out"]
    out += bo[None, None, :]
    if _run_kwargs:
        kernel.last_results = res
    return out
